# revision 1
# baseline (speedup 1.0000x reference)
"""Trainium2 Bass kernel for the HOI relation model.

Pipeline per core (2 images each, 8 cores data-parallel over batch):
  1. ROI mean pooling: pooled[d,c] = (1/area_d) * sum_hw mask[d,hw] * F[hw,c]
     computed as 32 K-chunk matmuls (mask stationary [128,32], features
     moving [128,768] in two N=384 halves), bf16 operands, f32 PSUM.
  2. PE-transpose pooled [32,768] -> pooledT [768,32] (6 transposes).
  3. Layer 1 factorized: relu(pair(h,o) @ w1 + b1) = relu(A(h) + B(o) + b1)
     where A = w1[:768].T @ h, B = w1[768:].T @ o  -- the 8x24 pair
     expansion happens AFTER the matmul (broadcast add on DVE).
  4. Layers 2, 3 as plain matmuls on the 384 pair rows (transposed layout).

Host does only O(B*D) prep: box->mask rasterization, score argsort
(baked into mask column order), 1/area, dtype casts, shard/gather.
"""

import numpy as np
import ml_dtypes

import concourse.bass as bass
import concourse.mybir as mybir
import concourse.tile as tile
from concourse import bacc
from concourse.bass_utils import run_bass_kernel_spmd
from concourse.masks import make_identity

N_CORES = 8
B, D, C = 16, 32, 768
NH, NO = 8, 24
NPAIR = NH * NO              # 192 pairs per image
GRID = 64                    # feature grid (896 / 14)
KPIX = GRID * GRID           # 4096 pixels per image
BL = B // N_CORES            # 2 images per core
KCH = KPIX // 128            # 32 K-chunks per image
CG = 4                       # K-chunks per DMA tile
H1, H2, H3 = 512, 256, 117
M = BL * NPAIR               # 384 pair rows per core

F32 = mybir.dt.float32
BF16 = mybir.dt.bfloat16
BF = ml_dtypes.bfloat16

_PROGRAM = None


def _build_program():
    nc = bacc.Bacc("TRN2", target_bir_lowering=False, debug=False,
                   num_devices=N_CORES)
    feat = nc.declare_dram_parameter("feat", [BL, KPIX, C], BF16, isOutput=False)
    maskT = nc.declare_dram_parameter("maskT", [BL, KPIX, D], BF16, isOutput=False)
    inva = nc.declare_dram_parameter("inva", [BL, D], F32, isOutput=False)
    w1 = nc.declare_dram_parameter("w1", [2 * C, H1], BF16, isOutput=False)
    b1 = nc.declare_dram_parameter("b1", [H1], F32, isOutput=False)
    w2 = nc.declare_dram_parameter("w2", [H1, H2], BF16, isOutput=False)
    b2 = nc.declare_dram_parameter("b2", [H2], F32, isOutput=False)
    w3 = nc.declare_dram_parameter("w3", [H2, H3], BF16, isOutput=False)
    b3 = nc.declare_dram_parameter("b3", [H3], F32, isOutput=False)
    out = nc.declare_dram_parameter("out", [M, H3], F32, isOutput=True)

    add = mybir.AluOpType.add
    amax = mybir.AluOpType.max

    with tile.TileContext(nc) as tc:
        with (
            tc.tile_pool(name="singles", bufs=1) as singles,
            tc.tile_pool(name="featp", bufs=6) as featp,
            tc.tile_pool(name="maskp", bufs=6) as maskp,
            tc.tile_pool(name="work", bufs=1) as work,
            tc.tile_pool(name="tmp", bufs=3) as tmpp,
            tc.tile_pool(name="pps", bufs=1, space="PSUM") as pps,
            tc.tile_pool(name="mps", bufs=4, space="PSUM") as mps,
        ):
            # ---- one-time constant loads ----
            ident = singles.tile([32, 32], BF16, tag="ident")
            make_identity(nc, ident)
            w1_sb = singles.tile([128, 12, H1], BF16, tag="w1")
            nc.sync.dma_start(out=w1_sb, in_=w1[:, :].rearrange("(kc p) n -> p kc n", p=128))
            w2_sb = singles.tile([128, 4, H2], BF16, tag="w2")
            nc.sync.dma_start(out=w2_sb, in_=w2[:, :].rearrange("(kc p) n -> p kc n", p=128))
            w3_sb = singles.tile([128, 2, H3], BF16, tag="w3")
            nc.sync.dma_start(out=w3_sb, in_=w3[:, :].rearrange("(kc p) n -> p kc n", p=128))
            b1_sb = singles.tile([128, 4], F32, tag="b1")
            nc.sync.dma_start(out=b1_sb, in_=b1[:].rearrange("(mc p) -> p mc", p=128))
            b2_sb = singles.tile([128, 2], F32, tag="b2")
            nc.sync.dma_start(out=b2_sb, in_=b2[:].rearrange("(mc p) -> p mc", p=128))
            b3_sb = singles.tile([128, H3], F32, tag="b3")
            b3_bcast = bass.AP(tensor=b3[:].tensor, offset=b3[:].offset,
                               ap=[[0, 128], [1, H3]])
            nc.sync.dma_start(out=b3_sb, in_=b3_bcast)
            inva_sb = singles.tile([D, BL], F32, tag="inva")
            nc.sync.dma_start(out=inva_sb, in_=inva[:, :].rearrange("b d -> d b"))

            # persistent activations
            pooledT = work.tile([128, BL, 6, D], BF16, tag="pooledT")
            x1T = work.tile([128, 4, M], BF16, tag="x1T")
            x2T = work.tile([128, 2, M], BF16, tag="x2T")

            # ---- pooling + transpose per image ----
            for img in range(BL):
                ps_a = pps.tile([D, 384], F32, tag=f"pp{img}a")
                ps_b = pps.tile([D, 384], F32, tag=f"pp{img}b")
                for g in range(KCH // CG):
                    f_sb = featp.tile([128, CG, C], BF16, tag="f")
                    nc.sync.dma_start(
                        out=f_sb,
                        in_=feat[img, g * CG * 128:(g + 1) * CG * 128, :]
                        .rearrange("(gc p) c -> p gc c", p=128))
                    m_sb = maskp.tile([128, CG, D], BF16, tag="m")
                    nc.sync.dma_start(
                        out=m_sb,
                        in_=maskT[img, g * CG * 128:(g + 1) * CG * 128, :]
                        .rearrange("(gc p) d -> p gc d", p=128))
                    for gc in range(CG):
                        kk = g * CG + gc
                        nc.tensor.matmul(ps_a, m_sb[:, gc, :], f_sb[:, gc, 0:384],
                                         start=(kk == 0), stop=(kk == KCH - 1))
                        nc.tensor.matmul(ps_b, m_sb[:, gc, :], f_sb[:, gc, 384:768],
                                         start=(kk == 0), stop=(kk == KCH - 1))
                # scale by 1/area, cast to bf16
                pooled = tmpp.tile([D, C], BF16, tag="pooled")
                nc.vector.tensor_scalar_mul(pooled[:, 0:384], ps_a, inva_sb[:, img:img + 1])
                nc.vector.tensor_scalar_mul(pooled[:, 384:768], ps_b, inva_sb[:, img:img + 1])
                # transpose to [C, D] in 6 chunks of 128 channels
                for cc in range(6):
                    ps_t = mps.tile([128, D], BF16, tag="mm")
                    nc.tensor.transpose(ps_t, pooled[:, cc * 128:(cc + 1) * 128], ident)
                    nc.vector.tensor_copy(pooledT[:, img, cc, :], ps_t)

            # ---- layer 1 (factorized over pairs) ----
            for mc in range(4):
                ps_ab = mps.tile([128, BL, D], F32, tag="mm")
                for kc in range(6):
                    nc.tensor.matmul(ps_ab[:, :, 0:NH],
                                     w1_sb[:, kc, mc * 128:(mc + 1) * 128],
                                     pooledT[:, :, kc, 0:NH],
                                     start=(kc == 0), stop=(kc == 5))
                for kc in range(6):
                    nc.tensor.matmul(ps_ab[:, :, NH:D],
                                     w1_sb[:, 6 + kc, mc * 128:(mc + 1) * 128],
                                     pooledT[:, :, kc, NH:D],
                                     start=(kc == 0), stop=(kc == 5))
                ab_sb = tmpp.tile([128, BL, D], F32, tag="ab")
                nc.vector.tensor_copy(ab_sb, ps_ab)
                for img in range(BL):
                    pre = tmpp.tile([128, NH, NO], F32, tag="pre")
                    a_bc = ab_sb[:, img, 0:NH][:, :, None].broadcast_to([128, NH, NO])
                    b_bc = ab_sb[:, img, NH:D][:, None, :].broadcast_to([128, NH, NO])
                    # pre = (A + b1) + B
                    nc.vector.scalar_tensor_tensor(pre, a_bc, b1_sb[:, mc:mc + 1],
                                                   b_bc, op0=add, op1=add)
                    dst = x1T[:, mc, img * NPAIR:(img + 1) * NPAIR] \
                        .rearrange("p (i j) -> p i j", i=NH)
                    nc.vector.tensor_scalar_max(dst, pre, 0.0)

            # ---- layer 2 ----
            for m2 in range(2):
                ps2 = mps.tile([128, M], F32, tag="mm")
                for kc in range(4):
                    nc.tensor.matmul(ps2, w2_sb[:, kc, m2 * 128:(m2 + 1) * 128],
                                     x1T[:, kc, :], start=(kc == 0), stop=(kc == 3))
                nc.vector.tensor_scalar(x2T[:, m2, :], ps2, b2_sb[:, m2:m2 + 1], 0.0,
                                        op0=add, op1=amax)

            # ---- layer 3 + bias + store ----
            for m3 in range(3):
                ps3 = mps.tile([128, H3], F32, tag="mm")
                for kc in range(2):
                    nc.tensor.matmul(ps3, x2T[:, kc, m3 * 128:(m3 + 1) * 128],
                                     w3_sb[:, kc, :], start=(kc == 0), stop=(kc == 1))
                o_sb = tmpp.tile([128, H3], F32, tag="osb")
                nc.vector.tensor_tensor(o_sb, ps3, b3_sb, op=add)
                nc.sync.dma_start(out=out[m3 * 128:(m3 + 1) * 128, :], in_=o_sb)
    nc.compile()
    return nc


def _get_program():
    global _PROGRAM
    if _PROGRAM is None:
        _PROGRAM = _build_program()
    return _PROGRAM


def _preprocess(boxes, scores):
    """Rasterize boxes to 0/1 masks with detection columns in sorted order."""
    cx, cy, bw, bh = boxes[..., 0], boxes[..., 1], boxes[..., 2], boxes[..., 3]
    x1 = np.floor((cx - bw / 2) * GRID).astype(np.int64)
    y1 = np.floor((cy - bh / 2) * GRID).astype(np.int64)
    x2 = np.floor((cx + bw / 2) * GRID).astype(np.int64)
    y2 = np.floor((cy + bh / 2) * GRID).astype(np.int64)
    hidx = np.argsort(-scores[:, :NH], axis=1, kind="stable")
    oidx = np.argsort(-scores[:, NH:], axis=1, kind="stable") + NH
    perm = np.concatenate([hidx, oidx], axis=1)                     # [B, D]
    g = np.arange(GRID)
    rows = (g[None, None, :] >= y1[..., None]) & (g[None, None, :] < y2[..., None])
    cols = (g[None, None, :] >= x1[..., None]) & (g[None, None, :] < x2[..., None])
    rows = np.take_along_axis(rows, perm[..., None], axis=1)        # [B, D, 64]
    cols = np.take_along_axis(cols, perm[..., None], axis=1)
    area = rows.sum(-1) * cols.sum(-1)                              # [B, D]
    mask = rows[:, :, :, None] & cols[:, :, None, :]                # [B, D, 64, 64]
    maskT = np.ascontiguousarray(
        mask.reshape(mask.shape[0], D, KPIX).transpose(0, 2, 1)).astype(BF)
    return maskT, (1.0 / area).astype(np.float32)


def _run(in_maps, trace=False, **kw):
    nc = _get_program()
    return run_bass_kernel_spmd(nc, in_maps, core_ids=list(range(N_CORES)),
                                trace=trace, **kw)


def _make_in_maps(features, boxes, scores, w1, b1, w2, b2, w3, b3):
    features = np.asarray(features, np.float32)
    maskT, inva = _preprocess(np.asarray(boxes, np.float32),
                              np.asarray(scores, np.float32))
    featb = np.ascontiguousarray(features.reshape(B, KPIX, C)).astype(BF)
    w1b = np.asarray(w1, np.float32).astype(BF)
    w2b = np.asarray(w2, np.float32).astype(BF)
    w3b = np.asarray(w3, np.float32).astype(BF)
    b1f = np.asarray(b1, np.float32)
    b2f = np.asarray(b2, np.float32)
    b3f = np.asarray(b3, np.float32)
    in_maps = []
    for c in range(N_CORES):
        s = slice(c * BL, (c + 1) * BL)
        in_maps.append({
            "feat": np.ascontiguousarray(featb[s]),
            "maskT": np.ascontiguousarray(maskT[s]),
            "inva": np.ascontiguousarray(inva[s]),
            "w1": w1b, "b1": b1f, "w2": w2b, "b2": b2f, "w3": w3b, "b3": b3f,
        })
    return in_maps


def kernel(features, boxes, scores, w1, b1, w2, b2, w3, b3, labels):
    in_maps = _make_in_maps(features, boxes, scores, w1, b1, w2, b2, w3, b3)
    res = _run(in_maps, trace=False)
    out = np.concatenate([r["out"].reshape(BL, NPAIR, H3) for r in res.results],
                         axis=0)
    return np.ascontiguousarray(out.astype(np.float32))



# revision 3
# speedup vs baseline: 1.5102x; 1.5102x over previous
"""Trainium2 Bass kernel for the HOI relation model.

Pipeline per core (2 images each, 8 cores data-parallel over batch):
  1. ROI mean pooling over a per-image crop window (union of all boxes,
     padded to NCH K-chunks of 128 pixels): pooled[d,c] = (1/area_d) *
     sum_k mask[k,d] * F[k,c], as NCH K-chunk matmuls (mask stationary
     [128,32], features moving [128,768] in two N=384 halves).
     Features and masks are fp8-e3m4 (masks are 0/1 -> exact); f32 PSUM.
  2. PE-transpose pooled [32,768] -> pooledT [768,32] (6 transposes).
  3. Layer 1 factorized: relu(pair(h,o) @ w1 + b1) = relu(A(h) + B(o) + b1)
     where A = w1[:768].T @ h, B = w1[768:].T @ o  -- the 8x24 pair
     expansion happens AFTER the matmul (broadcast add on DVE).
  4. Layers 2, 3 as plain matmuls on the 384 pair rows (transposed layout).

Host does only O(B*D)-ish prep: box->mask rasterization in the crop
window, score argsort (baked into mask column order), 1/area, dtype
casts and contiguous SBUF-layout packing, shard/gather.
"""

import numpy as np
import ml_dtypes

import concourse.bass as bass
import concourse.mybir as mybir
import concourse.tile as tile
from concourse import bacc
from concourse.bass_utils import run_bass_kernel_spmd
from concourse.masks import make_identity

N_CORES = 8
B, D, C = 16, 32, 768
NH, NO = 8, 24
NPAIR = NH * NO              # 192 pairs per image
GRID = 64                    # feature grid (896 / 14)
BL = B // N_CORES            # 2 images per core
CG = 5                       # K-chunks per feature DMA tile
H1, H2, H3 = 512, 256, 117
M = BL * NPAIR               # 384 pair rows per core

F32 = mybir.dt.float32
BF16 = mybir.dt.bfloat16
FP8 = mybir.dt.float8e3
BF = ml_dtypes.bfloat16
E3 = ml_dtypes.float8_e3m4

_PROGRAMS = {}


def _build_program(nch):
    ng = nch // CG
    nc = bacc.Bacc("TRN2", target_bir_lowering=False, debug=False,
                   num_devices=N_CORES)
    feat = nc.declare_dram_parameter("feat", [BL, ng, 128, CG, C], FP8,
                                     isOutput=False)
    maskT = nc.declare_dram_parameter("maskT", [BL, 128, nch, D], FP8,
                                      isOutput=False)
    inva = nc.declare_dram_parameter("inva", [D, BL], F32, isOutput=False)
    w1 = nc.declare_dram_parameter("w1", [128, 12, H1], BF16, isOutput=False)
    b1 = nc.declare_dram_parameter("b1", [128, 4], F32, isOutput=False)
    w2 = nc.declare_dram_parameter("w2", [128, 4, H2], BF16, isOutput=False)
    b2 = nc.declare_dram_parameter("b2", [128, 2], F32, isOutput=False)
    w3 = nc.declare_dram_parameter("w3", [128, 2, H3], BF16, isOutput=False)
    b3 = nc.declare_dram_parameter("b3", [128, H3], F32, isOutput=False)
    out = nc.declare_dram_parameter("out", [M, H3], F32, isOutput=True)

    add = mybir.AluOpType.add
    amax = mybir.AluOpType.max

    with tile.TileContext(nc) as tc:
        with (
            tc.tile_pool(name="singles", bufs=1) as singles,
            tc.tile_pool(name="featp", bufs=4) as featp,
            tc.tile_pool(name="work", bufs=1) as work,
            tc.tile_pool(name="tmp", bufs=3) as tmpp,
            tc.tile_pool(name="pps", bufs=1, space="PSUM") as pps,
            tc.tile_pool(name="mps", bufs=4, space="PSUM") as mps,
        ):
            # ---- masks first (small, needed by the first matmul) ----
            m_sb = []
            for img in range(BL):
                m = singles.tile([128, nch, D], FP8, tag=f"mask{img}")
                nc.sync.dma_start(out=m, in_=maskT[img])
                m_sb.append(m)
            ident = singles.tile([32, 32], BF16, tag="ident")
            make_identity(nc, ident)
            inva_sb = singles.tile([D, BL], F32, tag="inva")
            nc.sync.dma_start(out=inva_sb, in_=inva[:, :])

            # feature tiles for image 0 queued before the weights;
            # w1 queued before image 1's features so it lands in time
            # for layer 1 but does not delay image-0 pooling.
            f_tiles = [[None] * ng for _ in range(BL)]
            for g in range(ng):
                f = featp.tile([128, CG, C], FP8, tag="f")
                nc.sync.dma_start(out=f, in_=feat[0, g])
                f_tiles[0][g] = f
            w1_sb = singles.tile([128, 12, H1], BF16, tag="w1")
            nc.sync.dma_start(out=w1_sb, in_=w1[:, :, :])
            for g in range(ng):
                f = featp.tile([128, CG, C], FP8, tag="f")
                nc.sync.dma_start(out=f, in_=feat[1, g])
                f_tiles[1][g] = f
            w2_sb = singles.tile([128, 4, H2], BF16, tag="w2")
            nc.sync.dma_start(out=w2_sb, in_=w2[:, :, :])
            w3_sb = singles.tile([128, 2, H3], BF16, tag="w3")
            nc.sync.dma_start(out=w3_sb, in_=w3[:, :, :])
            b1_sb = singles.tile([128, 4], F32, tag="b1")
            nc.sync.dma_start(out=b1_sb, in_=b1[:, :])
            b2_sb = singles.tile([128, 2], F32, tag="b2")
            nc.sync.dma_start(out=b2_sb, in_=b2[:, :])
            b3_sb = singles.tile([128, H3], F32, tag="b3")
            nc.sync.dma_start(out=b3_sb, in_=b3[:, :])

            # persistent activations
            pooledT = work.tile([128, BL, 6, D], BF16, tag="pooledT")
            x1T = work.tile([128, 4, M], BF16, tag="x1T")
            x2T = work.tile([128, 2, M], BF16, tag="x2T")

            # ---- pooling + transpose per image ----
            for img in range(BL):
                ps_a = pps.tile([D, 384], F32, tag=f"pp{img}a")
                ps_b = pps.tile([D, 384], F32, tag=f"pp{img}b")
                for g in range(ng):
                    f_sb = f_tiles[img][g]
                    for gc in range(CG):
                        kk = g * CG + gc
                        nc.tensor.matmul(ps_a, m_sb[img][:, kk, :],
                                         f_sb[:, gc, 0:384],
                                         start=(kk == 0), stop=(kk == nch - 1))
                        nc.tensor.matmul(ps_b, m_sb[img][:, kk, :],
                                         f_sb[:, gc, 384:768],
                                         start=(kk == 0), stop=(kk == nch - 1))
                # scale by 1/area, cast to bf16
                pooled = tmpp.tile([D, C], BF16, tag="pooled")
                nc.vector.tensor_scalar_mul(pooled[:, 0:384], ps_a,
                                            inva_sb[:, img:img + 1])
                nc.vector.tensor_scalar_mul(pooled[:, 384:768], ps_b,
                                            inva_sb[:, img:img + 1])
                # transpose to [C, D] in 6 chunks of 128 channels
                for cc in range(6):
                    ps_t = mps.tile([128, D], BF16, tag="mm")
                    nc.tensor.transpose(ps_t, pooled[:, cc * 128:(cc + 1) * 128],
                                        ident)
                    nc.vector.tensor_copy(pooledT[:, img, cc, :], ps_t)

            # ---- layer 1 (factorized over pairs) ----
            for mc in range(4):
                ps_ab = mps.tile([128, BL, D], F32, tag="mm")
                for kc in range(6):
                    nc.tensor.matmul(ps_ab[:, :, 0:NH],
                                     w1_sb[:, kc, mc * 128:(mc + 1) * 128],
                                     pooledT[:, :, kc, 0:NH],
                                     start=(kc == 0), stop=(kc == 5))
                for kc in range(6):
                    nc.tensor.matmul(ps_ab[:, :, NH:D],
                                     w1_sb[:, 6 + kc, mc * 128:(mc + 1) * 128],
                                     pooledT[:, :, kc, NH:D],
                                     start=(kc == 0), stop=(kc == 5))
                ab_sb = tmpp.tile([128, BL, D], F32, tag="ab")
                nc.vector.tensor_copy(ab_sb, ps_ab)
                for img in range(BL):
                    pre = tmpp.tile([128, NH, NO], F32, tag="pre")
                    a_bc = ab_sb[:, img, 0:NH][:, :, None].broadcast_to(
                        [128, NH, NO])
                    b_bc = ab_sb[:, img, NH:D][:, None, :].broadcast_to(
                        [128, NH, NO])
                    # pre = (A + b1) + B
                    nc.vector.scalar_tensor_tensor(pre, a_bc, b1_sb[:, mc:mc + 1],
                                                   b_bc, op0=add, op1=add)
                    dst = x1T[:, mc, img * NPAIR:(img + 1) * NPAIR] \
                        .rearrange("p (i j) -> p i j", i=NH)
                    nc.vector.tensor_scalar_max(dst, pre, 0.0)

            # ---- layer 2 ----
            for m2 in range(2):
                ps2 = mps.tile([128, M], F32, tag="mm")
                for kc in range(4):
                    nc.tensor.matmul(ps2, w2_sb[:, kc, m2 * 128:(m2 + 1) * 128],
                                     x1T[:, kc, :], start=(kc == 0),
                                     stop=(kc == 3))
                nc.vector.tensor_scalar(x2T[:, m2, :], ps2, b2_sb[:, m2:m2 + 1],
                                        0.0, op0=add, op1=amax)

            # ---- layer 3 + bias + store ----
            for m3 in range(3):
                ps3 = mps.tile([128, H3], F32, tag="mm")
                for kc in range(2):
                    nc.tensor.matmul(ps3, x2T[:, kc, m3 * 128:(m3 + 1) * 128],
                                     w3_sb[:, kc, :], start=(kc == 0),
                                     stop=(kc == 1))
                o_sb = tmpp.tile([128, H3], F32, tag="osb")
                nc.vector.tensor_tensor(o_sb, ps3, b3_sb, op=add)
                nc.sync.dma_start(out=out[m3 * 128:(m3 + 1) * 128, :], in_=o_sb)
    nc.compile()
    return nc


def _get_program(nch):
    if nch not in _PROGRAMS:
        _PROGRAMS[nch] = _build_program(nch)
    return _PROGRAMS[nch]


def _preprocess(features, boxes, scores):
    """Crop per-image union window, rasterize 0/1 masks (detection columns
    in sorted-score order), pack features+masks into SBUF layout, e3m4."""
    Bc = features.shape[0]
    cx, cy, bw, bh = boxes[..., 0], boxes[..., 1], boxes[..., 2], boxes[..., 3]
    x1 = np.floor((cx - bw / 2) * GRID).astype(np.int64)
    y1 = np.floor((cy - bh / 2) * GRID).astype(np.int64)
    x2 = np.floor((cx + bw / 2) * GRID).astype(np.int64)
    y2 = np.floor((cy + bh / 2) * GRID).astype(np.int64)
    x1c, x2c = np.clip(x1, 0, GRID), np.clip(x2, 0, GRID)
    y1c, y2c = np.clip(y1, 0, GRID), np.clip(y2, 0, GRID)
    hidx = np.argsort(-scores[:, :NH], axis=1, kind="stable")
    oidx = np.argsort(-scores[:, NH:], axis=1, kind="stable") + NH
    perm = np.concatenate([hidx, oidx], axis=1)                     # [B, D]

    # per-image crop windows, one global (WR, WC) size
    WR = int((y2c.max(1) - y1c.min(1)).max())
    WC = int((x2c.max(1) - x1c.min(1)).max())
    y0 = np.minimum(y1c.min(1), GRID - WR)                          # [B]
    x0 = np.minimum(x1c.min(1), GRID - WC)
    npix = WR * WC
    nch = -(-npix // 128)                                           # ceil/128
    nch = -(-nch // CG) * CG                                        # mult of CG
    kwin = nch * 128
    ng = nch // CG

    g = np.arange(GRID)
    rows = (g[None, None, :] >= y1[..., None]) & (g[None, None, :] < y2[..., None])
    cols = (g[None, None, :] >= x1[..., None]) & (g[None, None, :] < x2[..., None])
    rows = np.take_along_axis(rows, perm[..., None], axis=1)        # [B, D, 64]
    cols = np.take_along_axis(cols, perm[..., None], axis=1)
    area = rows.sum(-1) * cols.sum(-1)                              # [B, D]

    featp = np.zeros((Bc, ng, 128, CG, C), dtype=E3)
    maskp = np.zeros((Bc, 128, nch, D), dtype=E3)
    for i in range(Bc):
        fwin = features[i, y0[i]:y0[i] + WR, x0[i]:x0[i] + WC, :]
        fflat = np.zeros((kwin, C), dtype=E3)
        fflat[:npix] = fwin.reshape(npix, C).astype(E3)
        featp[i] = fflat.reshape(ng, CG, 128, C).transpose(0, 2, 1, 3)
        rwin = rows[i][:, y0[i]:y0[i] + WR]                         # [D, WR]
        cwin = cols[i][:, x0[i]:x0[i] + WC]                         # [D, WC]
        mwin = (rwin[:, :, None] & cwin[:, None, :]).reshape(D, npix)
        mflat = np.zeros((kwin, D), dtype=E3)
        mflat[:npix] = mwin.T.astype(E3)
        maskp[i] = mflat.reshape(nch, 128, D).transpose(1, 0, 2)
    invaT = np.ascontiguousarray((1.0 / area).astype(np.float32).T)  # [D, B]
    return featp, maskp, invaT, nch


def _run(nch, in_maps, trace=False, **kw):
    nc = _get_program(nch)
    return run_bass_kernel_spmd(nc, in_maps, core_ids=list(range(N_CORES)),
                                trace=trace, **kw)


def _make_in_maps(features, boxes, scores, w1, b1, w2, b2, w3, b3):
    features = np.asarray(features, np.float32)
    featp, maskp, invaT, nch = _preprocess(
        features, np.asarray(boxes, np.float32), np.asarray(scores, np.float32))
    w1p = np.ascontiguousarray(
        np.asarray(w1, np.float32).astype(BF).reshape(12, 128, H1)
        .transpose(1, 0, 2))
    w2p = np.ascontiguousarray(
        np.asarray(w2, np.float32).astype(BF).reshape(4, 128, H2)
        .transpose(1, 0, 2))
    w3p = np.ascontiguousarray(
        np.asarray(w3, np.float32).astype(BF).reshape(2, 128, H3)
        .transpose(1, 0, 2))
    b1p = np.ascontiguousarray(np.asarray(b1, np.float32).reshape(4, 128).T)
    b2p = np.ascontiguousarray(np.asarray(b2, np.float32).reshape(2, 128).T)
    b3p = np.ascontiguousarray(
        np.broadcast_to(np.asarray(b3, np.float32), (128, H3)))
    in_maps = []
    for c in range(N_CORES):
        s = slice(c * BL, (c + 1) * BL)
        in_maps.append({
            "feat": np.ascontiguousarray(featp[s]),
            "maskT": np.ascontiguousarray(maskp[s]),
            "inva": np.ascontiguousarray(invaT[:, s]),
            "w1": w1p, "b1": b1p, "w2": w2p, "b2": b2p, "w3": w3p, "b3": b3p,
        })
    return in_maps, nch


def kernel(features, boxes, scores, w1, b1, w2, b2, w3, b3, labels):
    in_maps, nch = _make_in_maps(features, boxes, scores, w1, b1, w2, b2, w3, b3)
    res = _run(nch, in_maps, trace=False)
    out = np.concatenate([r["out"].reshape(BL, NPAIR, H3) for r in res.results],
                         axis=0)
    return np.ascontiguousarray(out.astype(np.float32))
